# revision 12
# baseline (speedup 1.0000x reference)
"""Trainium2 Bass kernel: BabyRadixAttention (GQA decode attention over a paged KV pool).

Problem shape (hardcoded per contract):
  B=32 requests, H=32 q heads, KH=8 kv heads, D=128, S=2049 tokens/request
  (2048 pool-resident + 1 new decode token written at out_cache_loc).

Sharding: data-parallel over the request batch -- core c handles requests
[4c, 4c+4). Each core reads its requests' contiguous slices of the flat
KV pool (the reference's req_to_token layout is contiguous arange), computes
full GQA attention for its 4 requests x 32 q-heads, and writes a [128, 128]
output block. Host reassembles to [32, 4096].

Device algorithm per core (all fp32; default emit is _emit_v2). Two hardware
constraints shape the layout: engine SBUF APs must start at partition
0/32/64/96 (so per-(req,head) work lands in free-dim slices at base
partition 0), and fp32 matmuls run as two half-speed passes with a
~333ns LDWEIGHTS per 128-column fp32 stationary (so the *small* operand is
always the stationary: 4-column weight loads).
  - load q [128(r,hq) x 128d], PE-transpose -> qT [d x rows]
  - stream K pool slices in 2MB rounds (512 tokens x 8 heads x 128d),
    PE-transpose 128x128 chunks -> KT (SBUF); per (req, round, head) one
    scores matmul [4 x 512] = (lhsT=qT 4 cols).T @ (rhs=KT [d x 512])
  - exp (ScalarE) [4 x 512] PSUM->SBUF, assembled into row-major p
    [128 rows x 2048] via SBUF->SBUF DMA (DMA may write any partition)
  - new-token scores via 32 tiny M=1 matmuls against kTnew columns
  - row sums: VectorE reduce over p + transposed new-token column;
    reciprocal kept for a final per-row scale
  - PE-transpose p chunks -> pT [s x rows]; PV per (req, round, head):
    4-matmul PSUM group (lhsT=pT slice [s x 4], rhs=V chunk [s x d]) ->
    [4 x 128] row-major, accumulated across rounds in SBUF (PSUM
    accumulation groups are bank-granular, so groups stay within a round)
  - output assembled with per-head SBUF->SBUF DMAs, scaled by 1/rowsum
    (per-partition tensor_scalar), one 64KB DMA out.
"""

import os

os.environ.setdefault("MYCRO_LOCAL_CACHE", "1")

import numpy as np

B, H, KH, D = 32, 32, 8, 128
G = H // KH                 # 4 q heads per kv head
S = 2049                    # tokens per request incl. new token
SC = S - 1                  # 2048 pool-resident tokens
N_CORES = 8
R = B // N_CORES            # 4 requests per core
RT = 512                    # tokens per DMA round
NROUND = SC // RT           # 4 rounds per request
NCH = RT // 128             # 4 128-token chunks per round
CH = 128

_PROG = {}


def _emit(ctx, tc, q_in, k_in, v_in, kb, vb, out_t):
    import concourse.bass as bass
    from concourse import masks, mybir

    nc = tc.nc
    f32 = mybir.dt.float32
    f32r = mybir.dt.float32r
    EXP = mybir.ActivationFunctionType.Exp
    scale = float(np.float32(1.0) / np.sqrt(np.float32(D)))

    # KERNEL_F32R=2 -> QK stage in fp32r: kt/qT/kTnew tiles carry fp32r
    # (the DVE copies producing them round), so scores matmuls run 1-pass.
    # Transposes, PV, and everything touching the output stay exact fp32.
    qkdt = f32r if os.environ.get("KERNEL_F32R", "0") == "2" else f32

    const_pool = ctx.enter_context(tc.tile_pool(name="const", bufs=1))
    ident = const_pool.tile([128, 128], f32)
    masks.make_identity(nc, ident[:])
    ones = const_pool.tile([128, 1], f32)
    nc.vector.memset(ones[:], 1.0)

    sb = ctx.enter_context(tc.tile_pool(name="sb", bufs=1))
    q_nat = sb.tile([R * H, D], f32)          # [128,128] rows (r, h*G+g)
    k_nat = sb.tile([R * KH, D], f32)         # [32,128] rows (r,h)
    vnew = sb.tile([1, R * KH * D], f32)      # new-token v, flat on one partition
    qT = sb.tile([128, R * H], qkdt)           # [d, rows]
    kTnew = sb.tile([128, R * KH], qkdt)       # [d, (r,h)]
    pT = sb.tile([128, SC], f32)              # [s-in-chunk, (chunk, row)]
    pT_last = sb.tile([1, 128], f32)          # new-token p row
    rsum_row = sb.tile([1, 128], f32)
    rs_col = sb.tile([128, 1], f32)
    rinv = sb.tile([128, 1], f32)
    outT_all = sb.tile([128, R * H], f32)     # [d, rows] PV accumulator
    out_s = sb.tile([R * H, D], f32)

    kstage = ctx.enter_context(tc.tile_pool(name="kstage", bufs=3))
    vstage = ctx.enter_context(tc.tile_pool(name="vstage", bufs=2))
    ktp = ctx.enter_context(tc.tile_pool(name="ktp", bufs=2))
    ps = ctx.enter_context(tc.tile_pool(name="ps", bufs=1, space="PSUM"))

    nc.sync.dma_start(q_nat[:], q_in[:])
    nc.sync.dma_start(k_nat[:], k_in[:])
    nc.sync.dma_start(vnew[:], v_in[:])

    # qT = q_nat.T ; kTnew = k_nat.T
    qT_ps = ps.tile([128, 128], f32, tag="tp", bufs=3)
    nc.tensor.transpose(qT_ps[:], q_nat[:], ident[:])
    nc.vector.tensor_copy(qT[:], qT_ps[:])
    kTn_ps = ps.tile([128, R * KH], f32, tag="tp", bufs=3)
    nc.tensor.transpose(kTn_ps[:], k_nat[:], ident[: R * KH, : R * KH])
    nc.vector.tensor_copy(kTnew[:], kTn_ps[:])

    # new-token scores^T: [1, 128] -- one M=1 matmul per (r,h) into free slices
    nsT_ps = ps.tile([1, 128], f32, tag="ns", bufs=1)
    for rh in range(R * KH):
        nc.tensor.matmul(
            nsT_ps[0:1, G * rh : G * rh + G],
            kTnew[:, rh : rh + 1],
            qT[:, G * rh : G * rh + G],
            start=True,
            stop=True,
        )
    nc.scalar.activation(pT_last[:], nsT_ps[:], EXP, scale=scale)

    # ---- QK phase ----
    for r in range(R):
        for ro in range(NROUND):
            ks = kstage.tile([128, NCH * KH * D], f32, tag="ks")
            tok0 = r * S + ro * RT
            src = kb[tok0 : tok0 + RT].rearrange("(c p) h d -> p c (h d)", p=128)
            nc.sync.dma_start(ks.rearrange("p (c hd) -> p c hd", c=NCH), src)
            kt = ktp.tile([128, KH * RT], qkdt, tag="kt")
            for c in range(NCH):
                for h in range(KH):
                    t_ps = ps.tile([128, 128], f32, tag="tp", bufs=3)
                    nc.tensor.transpose(
                        t_ps[:], ks[:, (c * KH + h) * D : (c * KH + h + 1) * D], ident[:]
                    )
                    nc.vector.tensor_copy(
                        kt[:, (c * KH + h) * CH : (c * KH + h + 1) * CH], t_ps[:]
                    )
            for c in range(NCH):
                sT_ps = ps.tile([128, H], f32, tag="st", bufs=3)
                for h in range(KH):
                    nc.tensor.matmul(
                        sT_ps[:, G * h : G * h + G],
                        kt[:, (c * KH + h) * CH : (c * KH + h + 1) * CH],
                        qT[:, r * H + G * h : r * H + G * h + G],
                        start=True,
                        stop=True,
                    )
                ch = ro * NCH + c
                nc.scalar.activation(
                    pT[:, ch * CH + r * H : ch * CH + (r + 1) * H],
                    sT_ps[:],
                    EXP,
                    scale=scale,
                )

    # ---- softmax row sums (over s) on PE via ones-vector ----
    rsum_ps = ps.tile([1, 128], f32, tag="rs", bufs=1)
    for ch in range(SC // CH):
        nc.tensor.matmul(
            rsum_ps[:],
            ones[:],
            pT[:, ch * CH : (ch + 1) * CH],
            start=(ch == 0),
            stop=False,
        )
    nc.tensor.matmul(
        rsum_ps[:], ones[0:1, 0:1], pT_last[:], start=False, stop=True
    )
    nc.vector.tensor_copy(rsum_row[:], rsum_ps[:])
    rsT_ps = ps.tile([128, 1], f32, tag="tp", bufs=3)
    nc.tensor.transpose(rsT_ps[:], rsum_row[:], ident[0:1, 0:1])
    nc.vector.tensor_copy(rs_col[:], rsT_ps[:])
    nc.vector.reciprocal(rinv[:], rs_col[:])

    # ---- PV phase ----
    # PSUM accumulation groups are bank-granular (2KB zero region): one group
    # at a time per bank. So each (r, ro, h) is a self-contained 4-matmul
    # group in its own bank slot; rounds accumulate in SBUF via VectorE adds.
    nc.vector.memset(outT_all[:], 0.0)
    for r in range(R):
        for ro in range(NROUND):
            vs = vstage.tile([128, NCH * KH * D], f32, tag="vs")
            tok0 = r * S + ro * RT
            src = vb[tok0 : tok0 + RT].rearrange("(c p) h d -> p c (h d)", p=128)
            nc.sync.dma_start(vs.rearrange("p (c hd) -> p c hd", c=NCH), src)
            for h in range(KH):
                ot_ps = ps.tile([128, G], f32, tag="st", bufs=3)
                for c in range(NCH):
                    ch = ro * NCH + c
                    nc.tensor.matmul(
                        ot_ps[:],
                        vs[:, (c * KH + h) * D : (c * KH + h + 1) * D],
                        pT[:, ch * CH + r * H + G * h : ch * CH + r * H + G * h + G],
                        start=(c == 0),
                        stop=(c == NCH - 1),
                    )
                col = r * H + G * h
                nc.vector.tensor_add(
                    outT_all[:, col : col + G], outT_all[:, col : col + G], ot_ps[:]
                )
        for h in range(KH):
            rh = r * KH + h
            nt_ps = ps.tile([128, G], f32, tag="st", bufs=3)
            nc.tensor.matmul(
                nt_ps[:],
                vnew[0:1, rh * D : (rh + 1) * D],
                pT_last[0:1, G * rh : G * rh + G],
                start=True,
                stop=True,
            )
            col = r * H + G * h
            nc.vector.tensor_add(
                outT_all[:, col : col + G], outT_all[:, col : col + G], nt_ps[:]
            )

    out_ps = ps.tile([128, 128], f32, tag="tp", bufs=3)
    nc.tensor.transpose(out_ps[:], outT_all[:], ident[:])
    nc.vector.tensor_scalar_mul(out_s[:], out_ps[:], rinv[:])
    nc.sync.dma_start(out_t[:], out_s[:])



def _emit_v2(ctx, tc, q_in, k_in, v_in, kb, vb, out_t):
    """Plan-A layout: QK as 128 wide-N matmuls [rows x s] with 4-column
    stationaries, row-major p assembled via SBUF->SBUF DMA, PE-transposed to
    pT for PV; PV uses pT-slice stationaries (4-col LDWEIGHTS) producing
    row-major [4 x 128] outputs accumulated in SBUF; final out assembled with
    one cross-partition SBUF->SBUF DMA and normalized per-row."""
    import concourse.bass as bass
    from concourse import masks, mybir

    nc = tc.nc
    f32 = mybir.dt.float32
    f32r = mybir.dt.float32r
    EXP = mybir.ActivationFunctionType.Exp
    AX = mybir.AxisListType.X
    scale = float(np.float32(1.0) / np.sqrt(np.float32(D)))
    qkdt = f32r if os.environ.get("KERNEL_F32R", "0") == "2" else f32

    const_pool = ctx.enter_context(tc.tile_pool(name="const", bufs=1))
    ident = const_pool.tile([128, 128], f32)
    masks.make_identity(nc, ident[:])

    sb = ctx.enter_context(tc.tile_pool(name="sb", bufs=1))
    q_nat = sb.tile([R * H, D], f32)
    k_nat = sb.tile([R * KH, D], f32)
    vnew = sb.tile([1, R * KH * D], f32)
    qT = sb.tile([128, R * H], qkdt)
    kTnew = sb.tile([128, R * KH], qkdt)
    p_rm = sb.tile([128, SC], f32)            # row-major p [rows, s]
    pT = sb.tile([128, SC], f32)              # [s-in-chunk, (chunk, row)]
    pT_last = sb.tile([1, 128], f32)
    ssum = sb.tile([128, 1], f32)
    nt_col = sb.tile([128, 1], f32)
    rinv = sb.tile([128, 1], f32)
    orow = sb.tile([G, R * KH * D], f32)      # [g, (rh, d)] PV accumulator
    outT2 = sb.tile([R * H, D], f32)
    out_s = sb.tile([R * H, D], f32)

    kstage = ctx.enter_context(tc.tile_pool(name="kstage", bufs=3))
    vstage = ctx.enter_context(tc.tile_pool(name="vstage", bufs=3))
    ktp = ctx.enter_context(tc.tile_pool(name="ktp", bufs=2))
    pe_pool = ctx.enter_context(tc.tile_pool(name="pe", bufs=4))
    ps = ctx.enter_context(tc.tile_pool(name="ps", bufs=1, space="PSUM"))

    nc.sync.dma_start(q_nat[:], q_in[:])
    nc.sync.dma_start(k_nat[:], k_in[:])
    nc.sync.dma_start(vnew[:], v_in[:])

    qT_ps = ps.tile([128, 128], f32, tag="tp", bufs=3)
    nc.tensor.transpose(qT_ps[:], q_nat[:], ident[:])
    nc.vector.tensor_copy(qT[:], qT_ps[:])
    kTn_ps = ps.tile([128, R * KH], f32, tag="tp", bufs=3)
    nc.tensor.transpose(kTn_ps[:], k_nat[:], ident[: R * KH, : R * KH])
    nc.vector.tensor_copy(kTnew[:], kTn_ps[:])

    nsT_ps = ps.tile([1, 128], f32, tag="ns", bufs=1)
    for rh in range(R * KH):
        nc.tensor.matmul(
            nsT_ps[0:1, G * rh : G * rh + G],
            kTnew[:, rh : rh + 1],
            qT[:, G * rh : G * rh + G],
            start=True,
            stop=True,
        )
    nc.scalar.activation(pT_last[:], nsT_ps[:], EXP, scale=scale)

    # ---- QK phase: one [4, 512] matmul per (r, ro, h) ----
    for r in range(R):
        for ro in range(NROUND):
            ks = kstage.tile([128, NCH * KH * D], f32, tag="ks")
            tok0 = r * S + ro * RT
            src = kb[tok0 : tok0 + RT].rearrange("(c p) h d -> p c (h d)", p=128)
            nc.sync.dma_start(ks.rearrange("p (c hd) -> p c hd", c=NCH), src)
            kt = ktp.tile([128, KH * RT], qkdt, tag="kt")
            for h in range(KH):
                t_ps = ps.tile([128, RT], f32, tag="tp", bufs=3)
                for c in range(NCH):
                    nc.tensor.transpose(
                        t_ps[:, c * CH : (c + 1) * CH],
                        ks[:, (c * KH + h) * D : (c * KH + h + 1) * D],
                        ident[:],
                    )
                nc.vector.tensor_copy(kt[:, h * RT : (h + 1) * RT], t_ps[:])
            for h in range(KH):
                rh = r * KH + h
                qk_ps = ps.tile([G, RT], f32, tag="qk", bufs=4)
                nc.tensor.matmul(
                    qk_ps[:],
                    qT[:, r * H + G * h : r * H + G * h + G],
                    kt[:, h * RT : (h + 1) * RT],
                    start=True,
                    stop=True,
                )
                pe_sb = pe_pool.tile([G, RT], f32, tag="pe")
                nc.scalar.activation(pe_sb[:], qk_ps[:], EXP, scale=scale)
                nc.sync.dma_start(
                    p_rm[G * rh : G * rh + G, ro * RT : (ro + 1) * RT], pe_sb[:]
                )

    # ---- softmax row sums + transpose p ----
    nc.vector.reduce_sum(ssum[:], p_rm[:], axis=AX)
    ntc_ps = ps.tile([128, 1], f32, tag="tp", bufs=3)
    nc.tensor.transpose(ntc_ps[:], pT_last[:], ident[0:1, 0:1])
    nc.vector.tensor_copy(nt_col[:], ntc_ps[:])
    nc.vector.tensor_add(ssum[:], ssum[:], nt_col[:])
    nc.vector.reciprocal(rinv[:], ssum[:])

    for ch in range(SC // CH):
        pt_ps = ps.tile([128, 128], f32, tag="tp", bufs=3)
        nc.tensor.transpose(pt_ps[:], p_rm[:, ch * CH : (ch + 1) * CH], ident[:])
        nc.vector.tensor_copy(pT[:, ch * CH : (ch + 1) * CH], pt_ps[:])

    # ---- PV phase: lhsT = pT slices (4-col LDW), rhs = V chunks ----
    nc.vector.memset(orow[:], 0.0)
    for r in range(R):
        for ro in range(NROUND):
            vs = vstage.tile([128, NCH * KH * D], f32, tag="vs")
            tok0 = r * S + ro * RT
            src = vb[tok0 : tok0 + RT].rearrange("(c p) h d -> p c (h d)", p=128)
            nc.sync.dma_start(vs.rearrange("p (c hd) -> p c hd", c=NCH), src)
            for h in range(KH):
                rh = r * KH + h
                o4 = ps.tile([G, D], f32, tag="qk", bufs=4)
                for c in range(NCH):
                    ch = ro * NCH + c
                    nc.tensor.matmul(
                        o4[:],
                        pT[:, ch * CH + r * H + G * h : ch * CH + r * H + G * h + G],
                        vs[:, (c * KH + h) * D : (c * KH + h + 1) * D],
                        start=(c == 0),
                        stop=(c == NCH - 1),
                    )
                nc.vector.tensor_add(
                    orow[:, rh * D : (rh + 1) * D], orow[:, rh * D : (rh + 1) * D], o4[:]
                )
        for h in range(KH):
            rh = r * KH + h
            nt_ps = ps.tile([G, D], f32, tag="qk", bufs=4)
            nc.tensor.matmul(
                nt_ps[:],
                pT_last[0:1, G * rh : G * rh + G],
                vnew[0:1, rh * D : (rh + 1) * D],
                start=True,
                stop=True,
            )
            nc.vector.tensor_add(
                orow[:, rh * D : (rh + 1) * D], orow[:, rh * D : (rh + 1) * D], nt_ps[:]
            )

    # assemble [rows, d] from [g, (rh, d)] via per-(r,h) SBUF->SBUF DMAs
    for rh in range(R * KH):
        nc.sync.dma_start(
            outT2[G * rh : G * rh + G, :], orow[:, rh * D : (rh + 1) * D]
        )
    nc.vector.tensor_scalar_mul(out_s[:], outT2[:], rinv[:])
    nc.sync.dma_start(out_t[:], out_s[:])



def _emit_v3(ctx, tc, q_in, k_in, v_in, kb, vb, out_t):
    """bf16 scoresT pipeline. Per request r (sequential, pipelined by Tile):

    QK: stage K rounds (fp32, 2MB); PE is_transpose on the bf16-bitcast high
    halves of each [128t, 128d] chunk (1cyc/row, exact truncation) -> kT
    chunks [d, t] in PSUM (bf16) -> DVE copy to SBUF kt. QK matmul per chunk:
    lhsT = kt chunk [d, 128t], rhs = qT 4 cols (r,h) -> scoresT [128t, 4]
    col-slices of one PSUM bank per (r, ro). One exp ACTIVATE [128, 128]
    PSUM->SBUF writes pT (bf16) via a 3D dst AP -- no p transposes, no
    row-major p assembly.

    Softmax: row sums via ones-vector matmuls over pT chunks (+ new-token),
    transpose [1,32]->[32,1], reciprocal into rinv rows r*32..+32.

    PV: stage V rounds, convert fp32->bf16 split Scalar/DVE. Per (half):
    PSUM bank [32rows, 512(h,d)] accumulates lhsT=pT chunk [128t, 32rows],
    rhs=vsb [128t, 512] over all 16 chunks, plus the new-token rank-1 matmul
    (lhsT=pT_last [1,32], rhs=vnew [1,512]) before stop. Valid output is
    block-diagonal: DVE copy bank->SBUF, then 4 small SBUF->SBUF DMAs pick
    [4rows, 128d] blocks into out_s. Final per-row 1/rowsum scale, one DMA out.
    """
    import concourse.bass as bass
    from concourse import masks, mybir

    nc = tc.nc
    f32 = mybir.dt.float32
    bf16 = mybir.dt.bfloat16
    EXP = mybir.ActivationFunctionType.Exp
    scale = float(np.float32(1.0) / np.sqrt(np.float32(D)))

    const_pool = ctx.enter_context(tc.tile_pool(name="const", bufs=1))
    ident = const_pool.tile([128, 128], bf16)
    masks.make_identity(nc, ident[:])
    identf = const_pool.tile([128, 128], f32)
    masks.make_identity(nc, identf[:])
    ones = const_pool.tile([128, 1], bf16)
    nc.vector.memset(ones[:], 1.0)

    sb = ctx.enter_context(tc.tile_pool(name="sb", bufs=1))
    q_nat = sb.tile([R * H, D], f32)          # [128,128] rows (r, h*G+g)
    k_nat = sb.tile([R * KH, D], f32)         # [32,128]
    vnew_f = sb.tile([1, R * KH * D], f32)    # [1, 4096]
    vnew = sb.tile([1, R * KH * D], bf16)
    qT = sb.tile([128, R * H], bf16)          # [d, rows]
    kTnew = sb.tile([128, R * KH], bf16)      # [d, (r,h)]
    rinv = sb.tile([128, 1], f32)
    out_s = sb.tile([R * H, D], f32)
    out_f = sb.tile([R * H, D], f32)

    # per-request tiles (rotated)
    kt_pool = ctx.enter_context(tc.tile_pool(name="kt", bufs=2))
    pt_pool = ctx.enter_context(tc.tile_pool(name="pt", bufs=2))
    kstage = ctx.enter_context(tc.tile_pool(name="ks", bufs=3))
    vstage = ctx.enter_context(tc.tile_pool(name="vs", bufs=3))
    vbst = ctx.enter_context(tc.tile_pool(name="vb", bufs=2))
    xst = ctx.enter_context(tc.tile_pool(name="xs", bufs=2))
    ps = ctx.enter_context(tc.tile_pool(name="ps", bufs=1, space="PSUM"))

    nc.sync.dma_start(q_nat[:], q_in[:])
    nc.sync.dma_start(k_nat[:], k_in[:])
    nc.sync.dma_start(vnew_f[:], v_in[:])
    nc.vector.tensor_copy(vnew[:], vnew_f[:])

    # qT / kTnew: fp32 PE transpose -> PSUM -> bf16 SBUF copies
    qT_ps = ps.tile([128, 512], f32, tag="sbk", bufs=2)
    nc.tensor.transpose(qT_ps[:, :128], q_nat[:], identf[:])
    nc.vector.tensor_copy(qT[:], qT_ps[:, :128])
    kTn_ps = ps.tile([128, 512], f32, tag="sbk", bufs=2)
    nc.tensor.transpose(kTn_ps[:, : R * KH], k_nat[:], identf[: R * KH, : R * KH])
    nc.vector.tensor_copy(kTnew[:], kTn_ps[:, : R * KH])

    NCHR = NROUND * NCH                       # 16 chunks per request

    for r in range(R):
        pT = pt_pool.tile([128, (NCHR + 1) * 32], bf16, tag="pt")

        # ---- QK ----
        for ro in range(NROUND):
            ks = kstage.tile([128, NCH * KH * D], f32, tag="ks")
            tok0 = r * S + ro * RT
            for c in range(NCH):
                nc.sync.dma_start(
                    ks[:, c * KH * D : (c + 1) * KH * D],
                    kb[tok0 + c * 128 : tok0 + (c + 1) * 128].rearrange(
                        "p h d -> p (h d)"
                    ),
                )
            ks_hi = (
                ks[:]
                .bitcast(bf16)
                .rearrange("p (ch d two) -> p ch d two", two=2, d=D)
            )
            kt = kt_pool.tile([128, NCH * KH * 128], bf16, tag="kt")
            for ti in range(4):
                ktp = ps.tile([128, 1024], bf16, tag="ktp", bufs=4)
                for sl in range(8):
                    ch = ti * 8 + sl
                    nc.tensor.transpose(
                        ktp[:, sl * 128 : (sl + 1) * 128], ks_hi[:, ch, :, 1], ident[:]
                    )
                nc.vector.tensor_copy(
                    kt[:, ti * 1024 : (ti + 1) * 1024], ktp[:]
                )
            sbk = ps.tile([128, 512], f32, tag="sbk", bufs=2)
            for c in range(NCH):
                for h in range(KH):
                    nc.tensor.matmul(
                        sbk[:, c * 32 + h * G : c * 32 + (h + 1) * G],
                        kt[:, (c * KH + h) * 128 : (c * KH + h + 1) * 128],
                        qT[:, r * H + G * h : r * H + G * h + G],
                        start=True,
                        stop=True,
                    )
            dst = pT[:].rearrange("p (ch x) -> p ch x", x=32)[
                :, ro * NCH : (ro + 1) * NCH, :
            ]
            nc.scalar.activation(
                dst,
                sbk[:, :128].rearrange("p (c x) -> p c x", x=32),
                EXP,
                scale=scale,
            )

        # ---- new-token scoresT ----
        ns_ps = ps.tile([128, 512], f32, tag="sbk", bufs=2)
        for h in range(KH):
            nc.tensor.matmul(
                ns_ps[0:1, h * G : (h + 1) * G],
                kTnew[:, r * KH + h : r * KH + h + 1],
                qT[:, r * H + G * h : r * H + G * h + G],
                start=True,
                stop=True,
            )
        nc.scalar.activation(
            pT[0:1, NCHR * 32 : (NCHR + 1) * 32], ns_ps[0:1, :32], EXP, scale=scale
        )

        # ---- row sums over t (ones-vector matmuls on pT chunks) ----
        rs_ps = ps.tile([128, 512], f32, tag="sbk", bufs=2)
        for ch in range(NCHR):
            nc.tensor.matmul(
                rs_ps[0:1, :32],
                ones[:],
                pT[:, ch * 32 : (ch + 1) * 32],
                start=(ch == 0),
                stop=False,
            )
        nc.tensor.matmul(
            rs_ps[0:1, :32],
            ones[0:1, 0:1],
            pT[0:1, NCHR * 32 : (NCHR + 1) * 32],
            start=False,
            stop=True,
        )
        rs_sb = xst.tile([1, 32], f32, tag="rssb")
        nc.vector.tensor_copy(rs_sb[:], rs_ps[0:1, :32])
        rsT_ps = ps.tile([128, 512], f32, tag="sbk", bufs=2)
        nc.tensor.transpose(rsT_ps[:32, :1], rs_sb[:], identf[0:1, 0:1])
        nc.vector.reciprocal(rinv[r * 32 : (r + 1) * 32, :], rsT_ps[:32, :1])

        # ---- PV ----
        pv0 = ps.tile([32, 512], f32, tag="pv0", bufs=1)
        pv1 = ps.tile([32, 512], f32, tag="pv1", bufs=1)
        for ro in range(NROUND):
            vs = vstage.tile([128, NCH * KH * D], f32, tag="vs")
            tok0 = r * S + ro * RT
            vsb = vbst.tile([128, NCH * KH * D], bf16, tag="vsb")
            for c in range(NCH):
                nc.sync.dma_start(
                    vs[:, c * KH * D : (c + 1) * KH * D],
                    vb[tok0 + c * 128 : tok0 + (c + 1) * 128].rearrange(
                        "p h d -> p (h d)"
                    ),
                )
                cvt = nc.scalar.copy if c % 2 == 0 else nc.vector.tensor_copy
                cvt(
                    vsb[:, c * KH * D : (c + 1) * KH * D],
                    vs[:, c * KH * D : (c + 1) * KH * D],
                )
                ch = ro * NCH + c
                for half, pv in ((0, pv0), (1, pv1)):
                    nc.tensor.matmul(
                        pv[:],
                        pT[:, ch * 32 : (ch + 1) * 32],
                        vsb[:, c * KH * D + half * 512 : c * KH * D + (half + 1) * 512],
                        start=(ro == 0 and c == 0),
                        stop=False,
                    )
        for half, pv in ((0, pv0), (1, pv1)):
            nc.tensor.matmul(
                pv[:],
                pT[0:1, NCHR * 32 : (NCHR + 1) * 32],
                vnew[0:1, r * KH * D + half * 512 : r * KH * D + (half + 1) * 512],
                start=False,
                stop=True,
            )
            stg = xst.tile([32, 512], f32, tag="stg")
            nc.vector.tensor_copy(stg[:], pv[:])
            for hh in range(4):
                h = half * 4 + hh
                nc.sync.dma_start(
                    out_s[r * H + h * G : r * H + (h + 1) * G, :],
                    stg[h * G : (h + 1) * G, hh * D : (hh + 1) * D],
                )

    nc.vector.tensor_scalar_mul(out_f[:], out_s[:], rinv[:])
    nc.sync.dma_start(out_t[:], out_f[:])


def _build_program():
    key = ("nc" + os.environ.get("KERNEL_F32R", "0")
           + os.environ.get("KERNEL_V", "3"))
    if key in _PROG:
        return _PROG[key]
    from contextlib import ExitStack

    import concourse.tile as tile
    from concourse import bacc, mybir

    f32 = mybir.dt.float32
    nc = bacc.Bacc("TRN2", target_bir_lowering=False, debug=False)
    q_in = nc.dram_tensor("q_in", (R * H, D), f32, kind="ExternalInput").ap()
    k_in = nc.dram_tensor("k_in", (R * KH, D), f32, kind="ExternalInput").ap()
    v_in = nc.dram_tensor("v_in", (1, R * KH * D), f32, kind="ExternalInput").ap()
    kb = nc.dram_tensor("kb", (R * S, KH, D), f32, kind="ExternalInput").ap()
    vb = nc.dram_tensor("vb", (R * S, KH, D), f32, kind="ExternalInput").ap()
    out_t = nc.dram_tensor("out", (R * H, D), f32, kind="ExternalOutput").ap()

    kv = os.environ.get("KERNEL_V", "3")
    emit = {"1": _emit, "2": _emit_v2}.get(kv, _emit_v3)
    with tile.TileContext(nc) as tc:
        with ExitStack() as ctx:
            emit(ctx, tc, q_in, k_in, v_in, kb, vb, out_t)
    nc.compile()
    _PROG[key] = nc
    return nc


def _inputs_match_expected_layout(req_to_token, req_pool_indices, out_cache_loc, seq_lens):
    if not np.array_equal(req_pool_indices, np.arange(B, dtype=req_pool_indices.dtype)):
        return False
    if not np.array_equal(seq_lens, np.full(B, S, dtype=seq_lens.dtype)):
        return False
    if not np.array_equal(
        req_to_token.ravel(), np.arange(B * S, dtype=req_to_token.dtype)
    ):
        return False
    if not np.array_equal(
        out_cache_loc, (np.arange(B, dtype=np.int64) * S + SC).astype(out_cache_loc.dtype)
    ):
        return False
    return True


def _numpy_fallback(q, k, v, k_buffer, v_buffer, req_to_token, req_pool_indices,
                    out_cache_loc, seq_lens):
    kb = np.array(k_buffer)
    vb = np.array(v_buffer)
    kb[out_cache_loc] = k
    vb[out_cache_loc] = v
    tok = req_to_token[req_pool_indices]
    out = np.empty((B, H, D), dtype=np.float32)
    scale = np.float32(1.0) / np.sqrt(np.float32(D))
    for b in range(B):
        sl = int(seq_lens[b])
        t = tok[b, :sl]
        k_all = kb[t]                       # [s, KH, D]
        v_all = vb[t]
        for hq in range(H):
            hk = hq // G
            s = (k_all[:, hk, :] @ q[b, hq]) * scale
            s = s - s.max()
            e = np.exp(s)
            pr = e / e.sum()
            out[b, hq] = pr @ v_all[:, hk, :]
    return out.reshape(B, H * D)


def kernel(q, k, v, k_buffer, v_buffer, req_to_token, req_pool_indices,
           out_cache_loc, seq_lens):
    q = np.asarray(q, dtype=np.float32)
    k = np.asarray(k, dtype=np.float32)
    v = np.asarray(v, dtype=np.float32)
    k_buffer = np.asarray(k_buffer, dtype=np.float32)
    v_buffer = np.asarray(v_buffer, dtype=np.float32)
    req_to_token = np.asarray(req_to_token)
    req_pool_indices = np.asarray(req_pool_indices)
    out_cache_loc = np.asarray(out_cache_loc)
    seq_lens = np.asarray(seq_lens)

    if not _inputs_match_expected_layout(
        req_to_token, req_pool_indices, out_cache_loc, seq_lens
    ):
        return _numpy_fallback(
            q, k, v, k_buffer, v_buffer, req_to_token, req_pool_indices,
            out_cache_loc, seq_lens,
        )

    from concourse.bass_utils import run_bass_kernel_spmd

    nc = _build_program()

    in_maps = []
    for c in range(N_CORES):
        rlo, rhi = R * c, R * (c + 1)
        in_maps.append(
            {
                "q_in": q[rlo:rhi].reshape(R * H, D),
                "k_in": k[rlo:rhi].reshape(R * KH, D),
                "v_in": v[rlo:rhi].reshape(1, R * KH * D),
                "kb": k_buffer[rlo * S : rhi * S],
                "vb": v_buffer[rlo * S : rhi * S],
            }
        )

    trace = bool(int(os.environ.get("KERNEL_TRACE", "0")))
    res = run_bass_kernel_spmd(nc, in_maps, core_ids=list(range(N_CORES)), trace=trace)
    if trace:
        _PROG["last_result"] = res

    out = np.concatenate([res.results[c]["out"] for c in range(N_CORES)], axis=0)
    return np.ascontiguousarray(out.reshape(B, H * D))



# revision 15
# speedup vs baseline: 1.0795x; 1.0795x over previous
"""Trainium2 Bass kernel: BabyRadixAttention (GQA decode attention over a paged KV pool).

Problem shape (hardcoded per contract):
  B=32 requests, H=32 q heads, KH=8 kv heads, D=128, S=2049 tokens/request
  (2048 pool-resident + 1 new decode token written at out_cache_loc).

Sharding: data-parallel over the request batch -- core c handles requests
[4c, 4c+4). Each core reads its requests' contiguous slices of the flat
KV pool (the reference's req_to_token layout is contiguous arange), computes
full GQA attention for its 4 requests x 32 q-heads, and writes a [128, 128]
output block. Host reassembles to [32, 4096].

Device algorithm per core (all fp32; default emit is _emit_v2). Two hardware
constraints shape the layout: engine SBUF APs must start at partition
0/32/64/96 (so per-(req,head) work lands in free-dim slices at base
partition 0), and fp32 matmuls run as two half-speed passes with a
~333ns LDWEIGHTS per 128-column fp32 stationary (so the *small* operand is
always the stationary: 4-column weight loads).
  - load q [128(r,hq) x 128d], PE-transpose -> qT [d x rows]
  - stream K pool slices in 2MB rounds (512 tokens x 8 heads x 128d),
    PE-transpose 128x128 chunks -> KT (SBUF); per (req, round, head) one
    scores matmul [4 x 512] = (lhsT=qT 4 cols).T @ (rhs=KT [d x 512])
  - exp (ScalarE) [4 x 512] PSUM->SBUF, assembled into row-major p
    [128 rows x 2048] via SBUF->SBUF DMA (DMA may write any partition)
  - new-token scores via 32 tiny M=1 matmuls against kTnew columns
  - row sums: VectorE reduce over p + transposed new-token column;
    reciprocal kept for a final per-row scale
  - PE-transpose p chunks -> pT [s x rows]; PV per (req, round, head):
    4-matmul PSUM group (lhsT=pT slice [s x 4], rhs=V chunk [s x d]) ->
    [4 x 128] row-major, accumulated across rounds in SBUF (PSUM
    accumulation groups are bank-granular, so groups stay within a round)
  - output assembled with per-head SBUF->SBUF DMAs, scaled by 1/rowsum
    (per-partition tensor_scalar), one 64KB DMA out.
"""

import os

os.environ.setdefault("MYCRO_LOCAL_CACHE", "1")

import numpy as np

B, H, KH, D = 32, 32, 8, 128
G = H // KH                 # 4 q heads per kv head
S = 2049                    # tokens per request incl. new token
SC = S - 1                  # 2048 pool-resident tokens
N_CORES = 8
R = B // N_CORES            # 4 requests per core
RT = 512                    # tokens per DMA round
NROUND = SC // RT           # 4 rounds per request
NCH = RT // 128             # 4 128-token chunks per round
CH = 128

_PROG = {}


def _emit(ctx, tc, q_in, k_in, v_in, kb, vb, out_t):
    import concourse.bass as bass
    from concourse import masks, mybir

    nc = tc.nc
    f32 = mybir.dt.float32
    f32r = mybir.dt.float32r
    EXP = mybir.ActivationFunctionType.Exp
    scale = float(np.float32(1.0) / np.sqrt(np.float32(D)))

    # KERNEL_F32R=2 -> QK stage in fp32r: kt/qT/kTnew tiles carry fp32r
    # (the DVE copies producing them round), so scores matmuls run 1-pass.
    # Transposes, PV, and everything touching the output stay exact fp32.
    qkdt = f32r if os.environ.get("KERNEL_F32R", "0") == "2" else f32

    const_pool = ctx.enter_context(tc.tile_pool(name="const", bufs=1))
    ident = const_pool.tile([128, 128], f32)
    masks.make_identity(nc, ident[:])
    ones = const_pool.tile([128, 1], f32)
    nc.vector.memset(ones[:], 1.0)

    sb = ctx.enter_context(tc.tile_pool(name="sb", bufs=1))
    q_nat = sb.tile([R * H, D], f32)          # [128,128] rows (r, h*G+g)
    k_nat = sb.tile([R * KH, D], f32)         # [32,128] rows (r,h)
    vnew = sb.tile([1, R * KH * D], f32)      # new-token v, flat on one partition
    qT = sb.tile([128, R * H], qkdt)           # [d, rows]
    kTnew = sb.tile([128, R * KH], qkdt)       # [d, (r,h)]
    pT = sb.tile([128, SC], f32)              # [s-in-chunk, (chunk, row)]
    pT_last = sb.tile([1, 128], f32)          # new-token p row
    rsum_row = sb.tile([1, 128], f32)
    rs_col = sb.tile([128, 1], f32)
    rinv = sb.tile([128, 1], f32)
    outT_all = sb.tile([128, R * H], f32)     # [d, rows] PV accumulator
    out_s = sb.tile([R * H, D], f32)

    kstage = ctx.enter_context(tc.tile_pool(name="kstage", bufs=3))
    vstage = ctx.enter_context(tc.tile_pool(name="vstage", bufs=2))
    ktp = ctx.enter_context(tc.tile_pool(name="ktp", bufs=2))
    ps = ctx.enter_context(tc.tile_pool(name="ps", bufs=1, space="PSUM"))

    nc.sync.dma_start(q_nat[:], q_in[:])
    nc.sync.dma_start(k_nat[:], k_in[:])
    nc.sync.dma_start(vnew[:], v_in[:])

    # qT = q_nat.T ; kTnew = k_nat.T
    qT_ps = ps.tile([128, 128], f32, tag="tp", bufs=3)
    nc.tensor.transpose(qT_ps[:], q_nat[:], ident[:])
    nc.vector.tensor_copy(qT[:], qT_ps[:])
    kTn_ps = ps.tile([128, R * KH], f32, tag="tp", bufs=3)
    nc.tensor.transpose(kTn_ps[:], k_nat[:], ident[: R * KH, : R * KH])
    nc.vector.tensor_copy(kTnew[:], kTn_ps[:])

    # new-token scores^T: [1, 128] -- one M=1 matmul per (r,h) into free slices
    nsT_ps = ps.tile([1, 128], f32, tag="ns", bufs=1)
    for rh in range(R * KH):
        nc.tensor.matmul(
            nsT_ps[0:1, G * rh : G * rh + G],
            kTnew[:, rh : rh + 1],
            qT[:, G * rh : G * rh + G],
            start=True,
            stop=True,
        )
    nc.scalar.activation(pT_last[:], nsT_ps[:], EXP, scale=scale)

    # ---- QK phase ----
    for r in range(R):
        for ro in range(NROUND):
            ks = kstage.tile([128, NCH * KH * D], f32, tag="ks")
            tok0 = r * S + ro * RT
            src = kb[tok0 : tok0 + RT].rearrange("(c p) h d -> p c (h d)", p=128)
            nc.sync.dma_start(ks.rearrange("p (c hd) -> p c hd", c=NCH), src)
            kt = ktp.tile([128, KH * RT], qkdt, tag="kt")
            for c in range(NCH):
                for h in range(KH):
                    t_ps = ps.tile([128, 128], f32, tag="tp", bufs=3)
                    nc.tensor.transpose(
                        t_ps[:], ks[:, (c * KH + h) * D : (c * KH + h + 1) * D], ident[:]
                    )
                    nc.vector.tensor_copy(
                        kt[:, (c * KH + h) * CH : (c * KH + h + 1) * CH], t_ps[:]
                    )
            for c in range(NCH):
                sT_ps = ps.tile([128, H], f32, tag="st", bufs=3)
                for h in range(KH):
                    nc.tensor.matmul(
                        sT_ps[:, G * h : G * h + G],
                        kt[:, (c * KH + h) * CH : (c * KH + h + 1) * CH],
                        qT[:, r * H + G * h : r * H + G * h + G],
                        start=True,
                        stop=True,
                    )
                ch = ro * NCH + c
                nc.scalar.activation(
                    pT[:, ch * CH + r * H : ch * CH + (r + 1) * H],
                    sT_ps[:],
                    EXP,
                    scale=scale,
                )

    # ---- softmax row sums (over s) on PE via ones-vector ----
    rsum_ps = ps.tile([1, 128], f32, tag="rs", bufs=1)
    for ch in range(SC // CH):
        nc.tensor.matmul(
            rsum_ps[:],
            ones[:],
            pT[:, ch * CH : (ch + 1) * CH],
            start=(ch == 0),
            stop=False,
        )
    nc.tensor.matmul(
        rsum_ps[:], ones[0:1, 0:1], pT_last[:], start=False, stop=True
    )
    nc.vector.tensor_copy(rsum_row[:], rsum_ps[:])
    rsT_ps = ps.tile([128, 1], f32, tag="tp", bufs=3)
    nc.tensor.transpose(rsT_ps[:], rsum_row[:], ident[0:1, 0:1])
    nc.vector.tensor_copy(rs_col[:], rsT_ps[:])
    nc.vector.reciprocal(rinv[:], rs_col[:])

    # ---- PV phase ----
    # PSUM accumulation groups are bank-granular (2KB zero region): one group
    # at a time per bank. So each (r, ro, h) is a self-contained 4-matmul
    # group in its own bank slot; rounds accumulate in SBUF via VectorE adds.
    nc.vector.memset(outT_all[:], 0.0)
    for r in range(R):
        for ro in range(NROUND):
            vs = vstage.tile([128, NCH * KH * D], f32, tag="vs")
            tok0 = r * S + ro * RT
            src = vb[tok0 : tok0 + RT].rearrange("(c p) h d -> p c (h d)", p=128)
            nc.sync.dma_start(vs.rearrange("p (c hd) -> p c hd", c=NCH), src)
            for h in range(KH):
                ot_ps = ps.tile([128, G], f32, tag="st", bufs=3)
                for c in range(NCH):
                    ch = ro * NCH + c
                    nc.tensor.matmul(
                        ot_ps[:],
                        vs[:, (c * KH + h) * D : (c * KH + h + 1) * D],
                        pT[:, ch * CH + r * H + G * h : ch * CH + r * H + G * h + G],
                        start=(c == 0),
                        stop=(c == NCH - 1),
                    )
                col = r * H + G * h
                nc.vector.tensor_add(
                    outT_all[:, col : col + G], outT_all[:, col : col + G], ot_ps[:]
                )
        for h in range(KH):
            rh = r * KH + h
            nt_ps = ps.tile([128, G], f32, tag="st", bufs=3)
            nc.tensor.matmul(
                nt_ps[:],
                vnew[0:1, rh * D : (rh + 1) * D],
                pT_last[0:1, G * rh : G * rh + G],
                start=True,
                stop=True,
            )
            col = r * H + G * h
            nc.vector.tensor_add(
                outT_all[:, col : col + G], outT_all[:, col : col + G], nt_ps[:]
            )

    out_ps = ps.tile([128, 128], f32, tag="tp", bufs=3)
    nc.tensor.transpose(out_ps[:], outT_all[:], ident[:])
    nc.vector.tensor_scalar_mul(out_s[:], out_ps[:], rinv[:])
    nc.sync.dma_start(out_t[:], out_s[:])



def _emit_v2(ctx, tc, q_in, k_in, v_in, kb, vb, out_t):
    """Plan-A layout: QK as 128 wide-N matmuls [rows x s] with 4-column
    stationaries, row-major p assembled via SBUF->SBUF DMA, PE-transposed to
    pT for PV; PV uses pT-slice stationaries (4-col LDWEIGHTS) producing
    row-major [4 x 128] outputs accumulated in SBUF; final out assembled with
    one cross-partition SBUF->SBUF DMA and normalized per-row."""
    import concourse.bass as bass
    from concourse import masks, mybir

    nc = tc.nc
    f32 = mybir.dt.float32
    f32r = mybir.dt.float32r
    EXP = mybir.ActivationFunctionType.Exp
    AX = mybir.AxisListType.X
    scale = float(np.float32(1.0) / np.sqrt(np.float32(D)))
    qkdt = f32r if os.environ.get("KERNEL_F32R", "0") == "2" else f32

    const_pool = ctx.enter_context(tc.tile_pool(name="const", bufs=1))
    ident = const_pool.tile([128, 128], f32)
    masks.make_identity(nc, ident[:])

    sb = ctx.enter_context(tc.tile_pool(name="sb", bufs=1))
    q_nat = sb.tile([R * H, D], f32)
    k_nat = sb.tile([R * KH, D], f32)
    vnew = sb.tile([1, R * KH * D], f32)
    qT = sb.tile([128, R * H], qkdt)
    kTnew = sb.tile([128, R * KH], qkdt)
    p_rm = sb.tile([128, SC], f32)            # row-major p [rows, s]
    pT = sb.tile([128, SC], f32)              # [s-in-chunk, (chunk, row)]
    pT_last = sb.tile([1, 128], f32)
    ssum = sb.tile([128, 1], f32)
    nt_col = sb.tile([128, 1], f32)
    rinv = sb.tile([128, 1], f32)
    orow = sb.tile([G, R * KH * D], f32)      # [g, (rh, d)] PV accumulator
    outT2 = sb.tile([R * H, D], f32)
    out_s = sb.tile([R * H, D], f32)

    kstage = ctx.enter_context(tc.tile_pool(name="kstage", bufs=3))
    vstage = ctx.enter_context(tc.tile_pool(name="vstage", bufs=3))
    ktp = ctx.enter_context(tc.tile_pool(name="ktp", bufs=2))
    pe_pool = ctx.enter_context(tc.tile_pool(name="pe", bufs=4))
    ps = ctx.enter_context(tc.tile_pool(name="ps", bufs=1, space="PSUM"))

    nc.sync.dma_start(q_nat[:], q_in[:])
    nc.sync.dma_start(k_nat[:], k_in[:])
    nc.sync.dma_start(vnew[:], v_in[:])

    qT_ps = ps.tile([128, 128], f32, tag="tp", bufs=3)
    nc.tensor.transpose(qT_ps[:], q_nat[:], ident[:])
    nc.vector.tensor_copy(qT[:], qT_ps[:])
    kTn_ps = ps.tile([128, R * KH], f32, tag="tp", bufs=3)
    nc.tensor.transpose(kTn_ps[:], k_nat[:], ident[: R * KH, : R * KH])
    nc.vector.tensor_copy(kTnew[:], kTn_ps[:])

    nsT_ps = ps.tile([1, 128], f32, tag="ns", bufs=1)
    for rh in range(R * KH):
        nc.tensor.matmul(
            nsT_ps[0:1, G * rh : G * rh + G],
            kTnew[:, rh : rh + 1],
            qT[:, G * rh : G * rh + G],
            start=True,
            stop=True,
        )
    nc.scalar.activation(pT_last[:], nsT_ps[:], EXP, scale=scale)

    # ---- QK phase: one [4, 512] matmul per (r, ro, h) ----
    for r in range(R):
        for ro in range(NROUND):
            ks = kstage.tile([128, NCH * KH * D], f32, tag="ks")
            tok0 = r * S + ro * RT
            src = kb[tok0 : tok0 + RT].rearrange("(c p) h d -> p c (h d)", p=128)
            nc.sync.dma_start(ks.rearrange("p (c hd) -> p c hd", c=NCH), src)
            kt = ktp.tile([128, KH * RT], qkdt, tag="kt")
            for h in range(KH):
                t_ps = ps.tile([128, RT], f32, tag="tp", bufs=3)
                for c in range(NCH):
                    nc.tensor.transpose(
                        t_ps[:, c * CH : (c + 1) * CH],
                        ks[:, (c * KH + h) * D : (c * KH + h + 1) * D],
                        ident[:],
                    )
                nc.vector.tensor_copy(kt[:, h * RT : (h + 1) * RT], t_ps[:])
            for h in range(KH):
                rh = r * KH + h
                qk_ps = ps.tile([G, RT], f32, tag="qk", bufs=4)
                nc.tensor.matmul(
                    qk_ps[:],
                    qT[:, r * H + G * h : r * H + G * h + G],
                    kt[:, h * RT : (h + 1) * RT],
                    start=True,
                    stop=True,
                )
                pe_sb = pe_pool.tile([G, RT], f32, tag="pe")
                nc.scalar.activation(pe_sb[:], qk_ps[:], EXP, scale=scale)
                nc.sync.dma_start(
                    p_rm[G * rh : G * rh + G, ro * RT : (ro + 1) * RT], pe_sb[:]
                )

    # ---- softmax row sums + transpose p ----
    nc.vector.reduce_sum(ssum[:], p_rm[:], axis=AX)
    ntc_ps = ps.tile([128, 1], f32, tag="tp", bufs=3)
    nc.tensor.transpose(ntc_ps[:], pT_last[:], ident[0:1, 0:1])
    nc.vector.tensor_copy(nt_col[:], ntc_ps[:])
    nc.vector.tensor_add(ssum[:], ssum[:], nt_col[:])
    nc.vector.reciprocal(rinv[:], ssum[:])

    for ch in range(SC // CH):
        pt_ps = ps.tile([128, 128], f32, tag="tp", bufs=3)
        nc.tensor.transpose(pt_ps[:], p_rm[:, ch * CH : (ch + 1) * CH], ident[:])
        nc.vector.tensor_copy(pT[:, ch * CH : (ch + 1) * CH], pt_ps[:])

    # ---- PV phase: lhsT = pT slices (4-col LDW), rhs = V chunks ----
    nc.vector.memset(orow[:], 0.0)
    for r in range(R):
        for ro in range(NROUND):
            vs = vstage.tile([128, NCH * KH * D], f32, tag="vs")
            tok0 = r * S + ro * RT
            src = vb[tok0 : tok0 + RT].rearrange("(c p) h d -> p c (h d)", p=128)
            nc.sync.dma_start(vs.rearrange("p (c hd) -> p c hd", c=NCH), src)
            for h in range(KH):
                rh = r * KH + h
                o4 = ps.tile([G, D], f32, tag="qk", bufs=4)
                for c in range(NCH):
                    ch = ro * NCH + c
                    nc.tensor.matmul(
                        o4[:],
                        pT[:, ch * CH + r * H + G * h : ch * CH + r * H + G * h + G],
                        vs[:, (c * KH + h) * D : (c * KH + h + 1) * D],
                        start=(c == 0),
                        stop=(c == NCH - 1),
                    )
                nc.vector.tensor_add(
                    orow[:, rh * D : (rh + 1) * D], orow[:, rh * D : (rh + 1) * D], o4[:]
                )
        for h in range(KH):
            rh = r * KH + h
            nt_ps = ps.tile([G, D], f32, tag="qk", bufs=4)
            nc.tensor.matmul(
                nt_ps[:],
                pT_last[0:1, G * rh : G * rh + G],
                vnew[0:1, rh * D : (rh + 1) * D],
                start=True,
                stop=True,
            )
            nc.vector.tensor_add(
                orow[:, rh * D : (rh + 1) * D], orow[:, rh * D : (rh + 1) * D], nt_ps[:]
            )

    # assemble [rows, d] from [g, (rh, d)] via per-(r,h) SBUF->SBUF DMAs
    for rh in range(R * KH):
        nc.sync.dma_start(
            outT2[G * rh : G * rh + G, :], orow[:, rh * D : (rh + 1) * D]
        )
    nc.vector.tensor_scalar_mul(out_s[:], outT2[:], rinv[:])
    nc.sync.dma_start(out_t[:], out_s[:])



def _emit_v3(ctx, tc, q_in, k_in, v_in, kb, vb, out_t):
    """bf16 scoresT pipeline. Per request r (sequential, pipelined by Tile):

    QK: stage K rounds (fp32, 2MB); PE is_transpose on the bf16-bitcast high
    halves of each [128t, 128d] chunk (1cyc/row, exact truncation) -> kT
    chunks [d, t] in PSUM (bf16) -> DVE copy to SBUF kt. QK matmul per chunk:
    lhsT = kt chunk [d, 128t], rhs = qT 4 cols (r,h) -> scoresT [128t, 4]
    col-slices of one PSUM bank per (r, ro). One exp ACTIVATE [128, 128]
    PSUM->SBUF writes pT (bf16) via a 3D dst AP -- no p transposes, no
    row-major p assembly.

    Softmax: row sums via ones-vector matmuls over pT chunks (+ new-token),
    transpose [1,32]->[32,1], reciprocal into rinv rows r*32..+32.

    PV: stage V rounds, convert fp32->bf16 split Scalar/DVE. Per (half):
    PSUM bank [32rows, 512(h,d)] accumulates lhsT=pT chunk [128t, 32rows],
    rhs=vsb [128t, 512] over all 16 chunks, plus the new-token rank-1 matmul
    (lhsT=pT_last [1,32], rhs=vnew [1,512]) before stop. Valid output is
    block-diagonal: DVE copy bank->SBUF, then 4 small SBUF->SBUF DMAs pick
    [4rows, 128d] blocks into out_s. Final per-row 1/rowsum scale, one DMA out.
    """
    import concourse.bass as bass
    from concourse import masks, mybir

    nc = tc.nc
    f32 = mybir.dt.float32
    bf16 = mybir.dt.bfloat16
    EXP = mybir.ActivationFunctionType.Exp
    scale = float(np.float32(1.0) / np.sqrt(np.float32(D)))

    const_pool = ctx.enter_context(tc.tile_pool(name="const", bufs=1))
    ident = const_pool.tile([128, 128], bf16)
    masks.make_identity(nc, ident[:])
    identf = const_pool.tile([128, 128], f32)
    masks.make_identity(nc, identf[:])
    ones = const_pool.tile([128, 1], bf16)
    nc.vector.memset(ones[:], 1.0)

    sb = ctx.enter_context(tc.tile_pool(name="sb", bufs=1))
    q_nat = sb.tile([R * H, D], f32)          # [128,128] rows (r, h*G+g)
    k_nat = sb.tile([R * KH, D], f32)         # [32,128]
    vnew_f = sb.tile([1, R * KH * D], f32)    # [1, 4096]
    vnew = sb.tile([1, R * KH * D], bf16)
    qT = sb.tile([128, R * H], bf16)          # [d, rows]
    kTnew = sb.tile([128, R * KH], bf16)      # [d, (r,h)]
    rinv = sb.tile([128, 1], f32)
    out_s = sb.tile([R * H, D], f32)
    out_f = sb.tile([R * H, D], f32)

    # per-request tiles (rotated)
    kt_pool = ctx.enter_context(tc.tile_pool(name="kt", bufs=2))
    pt_pool = ctx.enter_context(tc.tile_pool(name="pt", bufs=2))
    kstage = ctx.enter_context(tc.tile_pool(name="ks", bufs=10))
    vstage = ctx.enter_context(tc.tile_pool(name="vs", bufs=10))
    vbst = ctx.enter_context(tc.tile_pool(name="vb", bufs=4))
    xst = ctx.enter_context(tc.tile_pool(name="xs", bufs=2))
    ps = ctx.enter_context(tc.tile_pool(name="ps", bufs=1, space="PSUM"))

    nc.sync.dma_start(q_nat[:], q_in[:])
    nc.sync.dma_start(k_nat[:], k_in[:])
    nc.sync.dma_start(vnew_f[:], v_in[:])
    nc.vector.tensor_copy(vnew[:], vnew_f[:])

    # qT / kTnew: fp32 PE transpose -> PSUM -> bf16 SBUF copies
    qT_ps = ps.tile([128, 512], f32, tag="sbk", bufs=2)
    nc.tensor.transpose(qT_ps[:, :128], q_nat[:], identf[:])
    nc.vector.tensor_copy(qT[:], qT_ps[:, :128])
    kTn_ps = ps.tile([128, 512], f32, tag="sbk", bufs=2)
    nc.tensor.transpose(kTn_ps[:, : R * KH], k_nat[:], identf[: R * KH, : R * KH])
    nc.vector.tensor_copy(kTnew[:], kTn_ps[:, : R * KH])

    NCHR = NROUND * NCH                       # 16 chunks per request

    for r in range(R):
        pT = pt_pool.tile([128, (NCHR + 1) * 32], bf16, tag="pt")

        # ---- QK ----
        for ro in range(NROUND):
            tok0 = r * S + ro * RT
            kt = kt_pool.tile([128, NCH * KH * 128], bf16, tag="kt")
            for c in range(NCH):
                ksc = kstage.tile([128, KH * D], f32, tag="ks")
                nc.sync.dma_start(
                    ksc[:],
                    kb[tok0 + c * 128 : tok0 + (c + 1) * 128].rearrange(
                        "p h d -> p (h d)"
                    ),
                )
                ks_hi = (
                    ksc[:]
                    .bitcast(bf16)
                    .rearrange("p (h d two) -> p h d two", two=2, d=D)
                )
                ktp = ps.tile([128, 1024], bf16, tag="ktp", bufs=4)
                for h in range(KH):
                    nc.tensor.transpose(
                        ktp[:, h * 128 : (h + 1) * 128], ks_hi[:, h, :, 1], ident[:]
                    )
                nc.vector.tensor_copy(
                    kt[:, c * 1024 : (c + 1) * 1024], ktp[:]
                )
            sbk = ps.tile([128, 512], f32, tag="sbk", bufs=2)
            for c in range(NCH):
                for h in range(KH):
                    nc.tensor.matmul(
                        sbk[:, c * 32 + h * G : c * 32 + (h + 1) * G],
                        kt[:, (c * KH + h) * 128 : (c * KH + h + 1) * 128],
                        qT[:, r * H + G * h : r * H + G * h + G],
                        start=True,
                        stop=True,
                    )
            dst = pT[:].rearrange("p (ch x) -> p ch x", x=32)[
                :, ro * NCH : (ro + 1) * NCH, :
            ]
            nc.scalar.activation(
                dst,
                sbk[:, :128].rearrange("p (c x) -> p c x", x=32),
                EXP,
                scale=scale,
            )

        # ---- new-token scoresT ----
        ns_ps = ps.tile([128, 512], f32, tag="sbk", bufs=2)
        for h in range(KH):
            nc.tensor.matmul(
                ns_ps[0:1, h * G : (h + 1) * G],
                kTnew[:, r * KH + h : r * KH + h + 1],
                qT[:, r * H + G * h : r * H + G * h + G],
                start=True,
                stop=True,
            )
        nc.scalar.activation(
            pT[0:1, NCHR * 32 : (NCHR + 1) * 32], ns_ps[0:1, :32], EXP, scale=scale
        )

        # ---- row sums over t (ones-vector matmuls on pT chunks) ----
        rs_ps = ps.tile([128, 512], f32, tag="sbk", bufs=2)
        for ch in range(NCHR):
            nc.tensor.matmul(
                rs_ps[0:1, :32],
                ones[:],
                pT[:, ch * 32 : (ch + 1) * 32],
                start=(ch == 0),
                stop=False,
            )
        nc.tensor.matmul(
            rs_ps[0:1, :32],
            ones[0:1, 0:1],
            pT[0:1, NCHR * 32 : (NCHR + 1) * 32],
            start=False,
            stop=True,
        )
        rs_sb = xst.tile([1, 32], f32, tag="rssb")
        nc.vector.tensor_copy(rs_sb[:], rs_ps[0:1, :32])
        rsT_ps = ps.tile([128, 512], f32, tag="sbk", bufs=2)
        nc.tensor.transpose(rsT_ps[:32, :1], rs_sb[:], identf[0:1, 0:1])
        nc.vector.reciprocal(rinv[r * 32 : (r + 1) * 32, :], rsT_ps[:32, :1])

        # ---- PV ----
        pv0 = ps.tile([32, 512], f32, tag="pv0", bufs=1)
        pv1 = ps.tile([32, 512], f32, tag="pv1", bufs=1)
        for ro in range(NROUND):
            tok0 = r * S + ro * RT
            for c in range(NCH):
                vsc = vstage.tile([128, KH * D], f32, tag="vs")
                nc.sync.dma_start(
                    vsc[:],
                    vb[tok0 + c * 128 : tok0 + (c + 1) * 128].rearrange(
                        "p h d -> p (h d)"
                    ),
                )
                vsb = vbst.tile([128, KH * D], bf16, tag="vsb")
                cvt = nc.scalar.copy if c % 2 == 0 else nc.vector.tensor_copy
                cvt(vsb[:], vsc[:])
                ch = ro * NCH + c
                for half, pv in ((0, pv0), (1, pv1)):
                    nc.tensor.matmul(
                        pv[:],
                        pT[:, ch * 32 : (ch + 1) * 32],
                        vsb[:, half * 512 : (half + 1) * 512],
                        start=(ro == 0 and c == 0),
                        stop=False,
                    )
        for half, pv in ((0, pv0), (1, pv1)):
            nc.tensor.matmul(
                pv[:],
                pT[0:1, NCHR * 32 : (NCHR + 1) * 32],
                vnew[0:1, r * KH * D + half * 512 : r * KH * D + (half + 1) * 512],
                start=False,
                stop=True,
            )
            stg = xst.tile([32, 512], f32, tag="stg")
            nc.vector.tensor_copy(stg[:], pv[:])
            for hh in range(4):
                h = half * 4 + hh
                nc.sync.dma_start(
                    out_s[r * H + h * G : r * H + (h + 1) * G, :],
                    stg[h * G : (h + 1) * G, hh * D : (hh + 1) * D],
                )

    nc.vector.tensor_scalar_mul(out_f[:], out_s[:], rinv[:])
    nc.sync.dma_start(out_t[:], out_f[:])


def _build_program():
    key = ("nc" + os.environ.get("KERNEL_F32R", "0")
           + os.environ.get("KERNEL_V", "3"))
    if key in _PROG:
        return _PROG[key]
    from contextlib import ExitStack

    import concourse.tile as tile
    from concourse import bacc, mybir

    f32 = mybir.dt.float32
    nc = bacc.Bacc("TRN2", target_bir_lowering=False, debug=False)
    q_in = nc.dram_tensor("q_in", (R * H, D), f32, kind="ExternalInput").ap()
    k_in = nc.dram_tensor("k_in", (R * KH, D), f32, kind="ExternalInput").ap()
    v_in = nc.dram_tensor("v_in", (1, R * KH * D), f32, kind="ExternalInput").ap()
    kb = nc.dram_tensor("kb", (R * S, KH, D), f32, kind="ExternalInput").ap()
    vb = nc.dram_tensor("vb", (R * S, KH, D), f32, kind="ExternalInput").ap()
    out_t = nc.dram_tensor("out", (R * H, D), f32, kind="ExternalOutput").ap()

    kv = os.environ.get("KERNEL_V", "3")
    emit = {"1": _emit, "2": _emit_v2}.get(kv, _emit_v3)
    with tile.TileContext(nc) as tc:
        with ExitStack() as ctx:
            emit(ctx, tc, q_in, k_in, v_in, kb, vb, out_t)
    nc.compile()
    _PROG[key] = nc
    return nc


def _inputs_match_expected_layout(req_to_token, req_pool_indices, out_cache_loc, seq_lens):
    if not np.array_equal(req_pool_indices, np.arange(B, dtype=req_pool_indices.dtype)):
        return False
    if not np.array_equal(seq_lens, np.full(B, S, dtype=seq_lens.dtype)):
        return False
    if not np.array_equal(
        req_to_token.ravel(), np.arange(B * S, dtype=req_to_token.dtype)
    ):
        return False
    if not np.array_equal(
        out_cache_loc, (np.arange(B, dtype=np.int64) * S + SC).astype(out_cache_loc.dtype)
    ):
        return False
    return True


def _numpy_fallback(q, k, v, k_buffer, v_buffer, req_to_token, req_pool_indices,
                    out_cache_loc, seq_lens):
    kb = np.array(k_buffer)
    vb = np.array(v_buffer)
    kb[out_cache_loc] = k
    vb[out_cache_loc] = v
    tok = req_to_token[req_pool_indices]
    out = np.empty((B, H, D), dtype=np.float32)
    scale = np.float32(1.0) / np.sqrt(np.float32(D))
    for b in range(B):
        sl = int(seq_lens[b])
        t = tok[b, :sl]
        k_all = kb[t]                       # [s, KH, D]
        v_all = vb[t]
        for hq in range(H):
            hk = hq // G
            s = (k_all[:, hk, :] @ q[b, hq]) * scale
            s = s - s.max()
            e = np.exp(s)
            pr = e / e.sum()
            out[b, hq] = pr @ v_all[:, hk, :]
    return out.reshape(B, H * D)


def kernel(q, k, v, k_buffer, v_buffer, req_to_token, req_pool_indices,
           out_cache_loc, seq_lens):
    q = np.asarray(q, dtype=np.float32)
    k = np.asarray(k, dtype=np.float32)
    v = np.asarray(v, dtype=np.float32)
    k_buffer = np.asarray(k_buffer, dtype=np.float32)
    v_buffer = np.asarray(v_buffer, dtype=np.float32)
    req_to_token = np.asarray(req_to_token)
    req_pool_indices = np.asarray(req_pool_indices)
    out_cache_loc = np.asarray(out_cache_loc)
    seq_lens = np.asarray(seq_lens)

    if not _inputs_match_expected_layout(
        req_to_token, req_pool_indices, out_cache_loc, seq_lens
    ):
        return _numpy_fallback(
            q, k, v, k_buffer, v_buffer, req_to_token, req_pool_indices,
            out_cache_loc, seq_lens,
        )

    from concourse.bass_utils import run_bass_kernel_spmd

    nc = _build_program()

    in_maps = []
    for c in range(N_CORES):
        rlo, rhi = R * c, R * (c + 1)
        in_maps.append(
            {
                "q_in": q[rlo:rhi].reshape(R * H, D),
                "k_in": k[rlo:rhi].reshape(R * KH, D),
                "v_in": v[rlo:rhi].reshape(1, R * KH * D),
                "kb": k_buffer[rlo * S : rhi * S],
                "vb": v_buffer[rlo * S : rhi * S],
            }
        )

    trace = bool(int(os.environ.get("KERNEL_TRACE", "0")))
    res = run_bass_kernel_spmd(nc, in_maps, core_ids=list(range(N_CORES)), trace=trace)
    if trace:
        _PROG["last_result"] = res

    out = np.concatenate([res.results[c]["out"] for c in range(N_CORES)], axis=0)
    return np.ascontiguousarray(out.reshape(B, H * D))

